# revision 31
# baseline (speedup 1.0000x reference)
"""Trainium2 Bass kernel for nn_CrossAttentionBlock (B=4, C=512, H=W=64).

Core = (batch b, query-half h). Queries are split by (token mod 64):
half 0 owns tokens with n%64 in [0,32), half 1 owns [32,64). With the
torch-.view reinterpretation [B,N,C8]->[B,C8,H,W], viewed channel c maps
to y rows [64c, 64c+64); splitting on n%64 makes the W_y per-channel
instance stats a SUM of per-core Gram matrices:
  K[c,c'] = sum_{a,b} y[64c+a, b] y[64c'+a, b]   (a = n%64 within half)
so the only collective is a pairwise AllReduce of K_ext=[K|m] (64x65
fp32, ~17KB) instead of AllGather-ing y (256KB) and recomputing W_y.
  var_s(Cout) = w^T K w / N - (w^T m / N)^2,  mu_s = w^T m / N + W_b
out = r*x0 + t with r = sqrt((var_s+eps)/(var_c+eps)), t = mu_s - r*mu_c.

Per core:
  stage1: theta|phi = conv1x1(x1) with fp8 weights (x16 prescale), fp8 x1.
          Blocks 4-7 use a row-swapped stationary so theta lands on PSUM
          partitions 64-127 -> theta key-chunks 16-31 live at SBUF
          partitions 64-127 for true PE row-tiling. phi kept only for own
          (strided) queries, compacted; duplicated to both partition
          halves via SBUF-SBUF DMA.
  gT:     g^T token-major via x0(fp8)-chunk-stationary matmuls (FWL).
  main:   per key-chunk-pair p: f = theta^T phi for chunks p and p+16
          CONCURRENTLY (row tiles at partitions 0-63 / 64-127); exp on
          ACT (tile A) and Schraudolph-int16 on DVE (tile B); y^T
          accumulated in PSUM over 32 chunks with a ones column giving
          the softmax denominator. 1-pair lookahead pipeline.
  stats:  transpose y^T chunks -> normalize per query (+g_b) -> transpose
          back -> K via 32 strided matmuls -> AllReduce(add) 17KB ->
          KW matmul + reduce -> per-channel r,t -> out = r*x0h + t.
"""
import numpy as np
from contextlib import ExitStack

import concourse.bass as bass
import concourse.tile as tile
from concourse import mybir
from concourse.bass_utils import run_bass_kernel_spmd

FP32 = mybir.dt.float32
BF16 = mybir.dt.bfloat16
FP16 = mybir.dt.float16
FP8 = mybir.dt.float8e4
I16 = mybir.dt.int16
I32 = mybir.dt.int32
ALU = mybir.AluOpType
ACTF = mybir.ActivationFunctionType

B, C, H, W = 4, 512, 64, 64
N = H * W          # 4096 tokens
C8 = C // 8        # 64 inner channels
NQ = N // 2        # 2048 own queries per core
OC = C // 2        # 256 output channels per core
EPS = 1e-5
WS = 16.0          # fp8 weight prescale
IWS = 1.0 / WS

# Schraudolph exp in the bf16 domain: exp(x) ~= bitcast_bf16(int16(A*x+B))
EXPA = float((1 << 7) / np.log(2.0))
EXPB = float(127 * (1 << 7)) - 5.35

REPLICA_PAIRS = [[0, 1], [2, 3], [4, 5], [6, 7]]


def _split_excess_waits(nc, max_waits=1, drain_max=1):
    """walrus rejects instructions carrying more than ~2 sync waits; move
    extras to preceding NoOps on the same engine."""
    for blk in nc.main_func.blocks:
        insts = blk.instructions
        k = 0
        while k < len(insts):
            inst = insts[k]
            si = inst.sync_info
            cap = drain_max if inst.opcode == "Drain" else max_waits
            if si is not None and si.on_wait and len(si.on_wait) > cap:
                waits = list(si.on_wait)
                keep = waits[-cap:]
                extra = waits[:-cap]
                pos = k
                for j in range(0, len(extra), cap):
                    nop = mybir.InstNoOp(name=f"{inst.name}-wsplit{j}", ins=[], outs=[])
                    nop.engine = inst.engine
                    nop.sync_info = mybir.SyncInfo(
                        on_wait=extra[j : j + cap], on_update=[]
                    )
                    insts.insert(pos, nop)
                    pos += 1
                    k += 1
                inst.sync_info = mybir.SyncInfo(on_wait=keep, on_update=list(si.on_update))
            k += 1


def build_nc():
    """One SPMD program for all cores. Each core owns queries with
    n%64 in [0,32) of ITS (possibly group-rolled) token order; odd cores
    get x1/x0_8 rolled by 32 within each 64-token group on the host, so
    the kernel's strided-phi APs are core-independent."""
    h = 0
    nc = bass.Bass()

    x1_8 = nc.dram_tensor("x1_8", [C, N], FP8, kind="ExternalInput")
    x0_8 = nc.dram_tensor("x0_8", [C, N], FP8, kind="ExternalInput")
    x0h = nc.dram_tensor("x0h", [OC, N], FP16, kind="ExternalInput")
    tpw1 = nc.dram_tensor("tpw1", [C, 128], FP8, kind="ExternalInput")
    tpw2 = nc.dram_tensor("tpw2", [C, 128], FP8, kind="ExternalInput")
    tpb1 = nc.dram_tensor("tpb1", [128, 1], FP32, kind="ExternalInput")
    tpb2 = nc.dram_tensor("tpb2", [128, 1], FP32, kind="ExternalInput")
    gw8 = nc.dram_tensor("gw8", [C, C8], FP8, kind="ExternalInput")
    Wext = nc.dram_tensor("Wext", [C8 + 1, C], FP32, kind="ExternalInput")
    msel = nc.dram_tensor("msel", [128, 2], FP32, kind="ExternalInput")
    Wb2 = nc.dram_tensor("Wb2", [128, 2], FP32, kind="ExternalInput")
    gbc = nc.dram_tensor("gbc", [C8, 1], FP32, kind="ExternalInput")
    id65 = nc.dram_tensor("id65", [C8 + 1, C8 + 1], FP32, kind="ExternalInput")
    id128 = nc.dram_tensor("id128", [128, 128], FP32, kind="ExternalInput")
    out = nc.dram_tensor("out", [OC, N], FP16, kind="ExternalOutput")

    sin = nc.dram_tensor("sin", [2, 512], FP32)
    sout = nc.dram_tensor("sout", [2, 512], FP32)
    ccw_in = nc.dram_tensor("cc_warm_in", [1, 16], FP32)
    ccw_out = nc.dram_tensor("cc_warm_out", [1, 16], FP32)
    ccw2_in = nc.dram_tensor("ccw2_in", [1, 16], FP32)
    ccw2_out = nc.dram_tensor("ccw2_out", [1, 16], FP32)
    ccw3_in = nc.dram_tensor("ccw3_in", [1, 16], BF16)
    ccw3_out = nc.dram_tensor("ccw3_out", [1, 16], BF16)
    ccw4_in = nc.dram_tensor("ccw4_in", [1, 16], FP32)
    ccw4_out = nc.dram_tensor("ccw4_out", [1, 16], FP32)

    with tile.TileContext(nc) as tc, ExitStack() as ctx:
        wpool = ctx.enter_context(tc.tile_pool(name="weights", bufs=1))
        big = ctx.enter_context(tc.tile_pool(name="big", bufs=1))

        # ---- weight tiles ----
        tpw1_sb = wpool.tile([128, 4, 128], FP8)
        tpw2_sb = wpool.tile([128, 4, 128], FP8)
        gw_sb = wpool.tile([128, 4, C8], FP8)
        tpb1_sb = wpool.tile([128, 1], FP32)
        tpb2_sb = wpool.tile([128, 1], FP32)
        W_sb = wpool.tile([C8 + 1, C], FP32)
        ms_sb = wpool.tile([128, 2], FP32)
        Wb_sb = wpool.tile([128, 2], FP32)
        gb_sb = wpool.tile([C8, 1], FP32)
        id65_sb = wpool.tile([C8 + 1, C8 + 1], FP32)
        id128_sb = wpool.tile([128, 128], FP32)
        ones64 = wpool.tile([C8, 1], BF16)

        # ---- persistent big tensors (per-chunk tiles => subtile deps) ----
        x1c = [big.tile([128, N], FP8, name=f"x1c{c}") for c in range(4)]
        x0c = [big.tile([128, N], FP8, name=f"x0c{c}") for c in range(4)]
        x0h_sb = [big.tile([128, N], FP16, name=f"x0h{o}") for o in range(2)]

        # critical weights first (small), on all 3 queues
        for c in range(4):
            eng3 = [nc.sync, nc.scalar, nc.gpsimd][c % 3]
            eng3.dma_start(out=tpw1_sb[:, c, :], in_=tpw1[c * 128:(c + 1) * 128, :])
        nc.sync.dma_start(out=tpb1_sb[:], in_=tpb1[:])
        nc.scalar.dma_start(out=tpb2_sb[:], in_=tpb2[:])
        thAB = big.tile([128, 16, 128], BF16)   # theta; rows 0-63 chunks 0-15,
                                                # rows 64-127 chunks 16-31
        ph2 = big.tile([128, NQ], BF16)         # own-query phi, both halves
        g_ext = big.tile([128, 32, C8 + 2], BF16)  # gT chunks + ones col
        yraw = big.tile([C8 + 1, NQ], FP32)     # yT_ext (pre-normalization)
        ynx = big.tile([128, 16, C8], FP32)     # transposed normalized y
        yn = big.tile([C8, NQ + C8], BF16)      # channel-major y + ones cols

        # warm the exp table + CC stack early (before gpsimd's DMA pacing)
        warm = wpool.tile([128, 1], FP32)
        nc.scalar.activation(warm[:], tpb1_sb[:], ACTF.Exp)
        nc.gpsimd.memset(g_ext[:, :, C8:C8 + 1], 1.0)
        nc.gpsimd.memset(yn[:, NQ:NQ + 32], 1.0)
        nc.gpsimd.memset(ones64[:], 1.0)
        nc.gpsimd.collective_compute(
            "AllReduce", ALU.add,
            replica_groups=REPLICA_PAIRS,
            ins=[ccw_in[:]],
            outs=[ccw_out[:]],
        )

        # ---- input DMAs: x1 on all 3 queues (scalar's triggers drain
        # before ACT's first exp), everything else on sync+gpsimd ----
        eng3 = [nc.sync, nc.scalar, nc.gpsimd]
        k = 0
        for q in range(4):
            cols = slice(q * 1024, (q + 1) * 1024)
            for c in range(4):
                eng3[k % 3].dma_start(out=x1c[c][:, cols],
                                      in_=x1_8[c * 128:(c + 1) * 128, cols])
                k += 1
            if q == 0:
                # tpw2 needed from stage1 block 4 on
                for c in range(4):
                    eng3[(k + c) % 3].dma_start(
                        out=tpw2_sb[:, c, :], in_=tpw2[c * 128:(c + 1) * 128, :])
        eng2 = [nc.sync, nc.gpsimd]
        for c in range(4):
            eng2[c % 2].dma_start(out=gw_sb[:, c, :],
                                  in_=gw8[c * 128:(c + 1) * 128, :])
        k = 0
        for q in range(4):
            cols = slice(q * 1024, (q + 1) * 1024)
            for c in range(4):
                eng2[k % 2].dma_start(out=x0c[c][:, cols],
                                      in_=x0_8[c * 128:(c + 1) * 128, cols])
                k += 1
        for oc in range(2):
            eng2[oc % 2].dma_start(out=x0h_sb[oc][:],
                                   in_=x0h[oc * 128:(oc + 1) * 128, :])
        # late-needed small tensors at the queue tails
        nc.sync.dma_start(out=id65_sb[:], in_=id65[:])
        nc.gpsimd.dma_start(out=id128_sb[:], in_=id128[:])
        nc.gpsimd.dma_start(out=W_sb[:], in_=Wext[:])
        nc.gpsimd.dma_start(out=ms_sb[:], in_=msel[:])
        nc.sync.dma_start(out=Wb_sb[:], in_=Wb2[:])
        nc.sync.dma_start(out=gb_sb[:], in_=gbc[:])

        # ---- stage 1: x1 -> theta/phi (fp8 weights, x16 prescale) ----
        with tc.tile_pool(name="ps_tp", bufs=2, space="PSUM") as ps_tp:
            for b in range(8):
                cols = slice(b * 512, (b + 1) * 512)
                tpw_sb = tpw1_sb if b < 4 else tpw2_sb
                ptp = ps_tp.tile([128, 512], FP32, name="ptp")
                for c in range(4):
                    nc.tensor.matmul(ptp[:], tpw_sb[:, c, :], x1c[c][:, cols],
                                     start=(c == 0), stop=(c == 3))
                trows = slice(0, 64) if b < 4 else slice(64, 128)
                prows = slice(64, 128) if b < 4 else slice(0, 64)
                tpb_sb = tpb1_sb if b < 4 else tpb2_sb
                ch = (b % 4) * 4
                # theta (full block) on DVE: (psum * 1/16) + bias
                nc.vector.tensor_scalar(
                    thAB[trows, ch:ch + 4, :].rearrange("p a b -> p (a b)"),
                    ptp[trows, :], IWS, tpb_sb[trows, :],
                    ALU.mult, ALU.add)
                # phi (own strided queries, compacted) on DVE
                nc.vector.tensor_scalar(
                    ph2[prows, b * 256:(b + 1) * 256].rearrange(
                        "p (g k) -> p g k", k=32),
                    ptp[prows, :].rearrange("p (g k) -> p g k", k=64)[
                        :, :, 32 * h:32 * h + 32],
                    IWS, tpb_sb[prows, :], ALU.mult, ALU.add)

        # phi lives at rows 64-127 for blocks 0-3, rows 0-63 for blocks 4-7;
        # duplicate each half to the other partition range (SBUF-SBUF DMA)
        nc.scalar.dma_start(out=ph2[0:64, 0:1024], in_=ph2[64:128, 0:1024])
        nc.scalar.dma_start(out=ph2[64:128, 1024:2048], in_=ph2[0:64, 1024:2048])

        # ---- gT: x0 chunks stationary (fp8, FWL), gw moving ----
        with tc.tile_pool(name="ps_g", bufs=3, space="PSUM") as ps_g:
            for mj in range(16):
                pg = ps_g.tile([128, 2, C8], FP32, name="pg")
                for half in range(2):
                    mi = mj * 2 + half
                    for c in range(4):
                        nc.tensor.matmul(pg[:, half, :],
                                         x0c[c][:, mi * 128:(mi + 1) * 128],
                                         gw_sb[:, c, :],
                                         start=(c == 0), stop=(c == 3))
                nc.vector.tensor_scalar(
                    g_ext[:, mj * 2:mj * 2 + 2, 0:C8], pg[:], IWS, None,
                    ALU.mult)

        # ---- main attention loop ----
        stat = ctx.enter_context(tc.tile_pool(name="stats", bufs=1))
        xst = stat.tile([128, 2, 8, 6], FP32)
        xagg = stat.tile([128, 2, 2], FP32)

        def emit_fwd_T(j, ps_t, ystage):
            ptile = ps_t.tile([128, C8 + 1], FP32, name="ptile", tag="pt")
            nc.tensor.transpose(ptile[:], yraw[:, j * 128:(j + 1) * 128],
                                id65_sb[:])
            rec = ystage.tile([128, 1], FP32, name="rec", tag="rec")
            nc.vector.reciprocal(rec[:], ptile[:, C8:C8 + 1])
            nc.vector.tensor_scalar(ynx[:, j, :], ptile[:, 0:C8], rec[:],
                                    None, ALU.mult)

        def emit_back_T(j, ps_t2):
            pt2 = ps_t2.tile([C8, 128], FP32, name="pt2", tag="pt2")
            nc.tensor.transpose(pt2[:], ynx[:, j, :], id128_sb[:])
            nc.vector.tensor_scalar(
                yn[:, j * 128:(j + 1) * 128], pt2[:], gb_sb[:], None,
                ALU.add)

        with tc.tile_pool(name="ps_fA", bufs=3, space="PSUM") as ps_fA, \
             tc.tile_pool(name="ps_fB", bufs=3, space="PSUM") as ps_fB, \
             tc.tile_pool(name="ps_y", bufs=1, space="PSUM") as ps_y, \
             tc.tile_pool(name="ptA", bufs=4) as ptA_pool, \
             tc.tile_pool(name="ptB", bufs=4) as ptB_pool:
            for pss in range(2):
                py = ps_y.tile([C8 + 1, 2, 512], FP32, name="py")

                def emit_y(args):
                    p, pas, pbs = args
                    for s in range(2):
                        nc.tensor.matmul(py[:, s, :], g_ext[:, p, 0:C8 + 1],
                                         pas[s][:], start=(p == 0), stop=False)
                    for s in range(2):
                        nc.tensor.matmul(py[:, s, :], g_ext[:, p + 16, 0:C8 + 1],
                                         pbs[s][:].bitcast(BF16),
                                         start=False, stop=(p == 15))

                prev = None
                for p in range(16):
                    fas, fbs, pas, pbs = [], [], [], []
                    for s in range(2):
                        qs = slice(pss * 1024 + s * 512,
                                   pss * 1024 + (s + 1) * 512)
                        fa = ps_fA.tile([128, 512], FP32, name="fa", tag="fa")
                        nc.tensor.matmul(fa[:], thAB[0:64, p, :], ph2[0:64, qs],
                                         start=True, stop=True)
                        fas.append(fa)
                    for s in range(2):
                        qs = slice(pss * 1024 + s * 512,
                                   pss * 1024 + (s + 1) * 512)
                        fb = ps_fB.tile([128, 512], FP32, name="fb", tag="fb")
                        nc.tensor.matmul(fb[:], thAB[64:128, p, :],
                                         ph2[64:128, qs],
                                         start=True, stop=True)
                        fbs.append(fb)
                    for s in range(2):
                        pa = ptA_pool.tile([128, 512], BF16, name="pa", tag="pa")
                        nc.scalar.activation(pa[:], fas[s][:], ACTF.Exp)
                        pas.append(pa)
                        pb = ptB_pool.tile([128, 512], I16, name="pb", tag="pb")
                        nc.vector.tensor_scalar(pb[:], fbs[s][:], EXPA, EXPB,
                                                ALU.mult, ALU.add)
                        pbs.append(pb)
                    if prev is not None:
                        emit_y(prev)
                    prev = (p, pas, pbs)
                emit_y(prev)

                nc.vector.tensor_copy(
                    yraw[:, pss * 1024:(pss + 1) * 1024],
                    py[:].rearrange("p a b -> p (a b)"))
                if pss == 0:
                    nc.scalar.dma_start(out=ccw2_in[:],
                                        in_=yraw[C8:C8 + 1, 0:16])
                    nc.gpsimd.collective_compute(
                        "AllReduce", ALU.add,
                        replica_groups=REPLICA_PAIRS,
                        ins=[ccw2_in[:]], outs=[ccw2_out[:]])

        # ---- transpose yT chunks, normalize, transpose back ----
        with tc.tile_pool(name="ps_t", bufs=3, space="PSUM") as ps_t, \
             tc.tile_pool(name="ps_t2", bufs=3, space="PSUM") as ps_t2, \
             tc.tile_pool(name="ystage", bufs=4) as ystage:
            for j in range(16):
                emit_fwd_T(j, ps_t, ystage)
                emit_back_T(j, ps_t2)

        # ---- K = sum_a yn_a yn_a^T (+ mean col via ones), AllReduce ----
        yn_r = yn[:].rearrange("p (c a) -> p c a", a=32)
        with tc.tile_pool(name="ps_k", bufs=1, space="PSUM") as ps_k, \
             tc.tile_pool(name="kst", bufs=1) as kst:
            kps = ps_k.tile([C8, C8 + 1], FP32, name="kps")
            for a in range(32):
                nc.tensor.matmul(
                    kps[:],
                    yn_r[:, 0:C8, a:a + 1].rearrange("p c o -> p (c o)"),
                    yn_r[:, 0:C8 + 1, a:a + 1].rearrange("p c o -> p (c o)"),
                    start=(a == 0), stop=(a == 31))
            ksb = kst.tile([C8, C8 + 1], FP32, name="ksb")
            nc.vector.tensor_copy(ksb[:], kps[:])

            # ---- per-channel S1 = w^T K_own w, S2 = m_own^T w (linear in
            # K => AllReduce the [128,4] scalars instead of K itself) ----
            with tc.tile_pool(name="ps_kw", bufs=1, space="PSUM") as ps_kw, \
                 tc.tile_pool(name="sc", bufs=1) as sc:
                # partial S for ALL 512 out channels in GLOBAL order so
                # the pairwise AllReduce adds matching quadratic forms;
                # S1 = column sums of P via ones^T @ P, S2 = P row 64
                kw = ps_kw.tile([C8 + 1, C], FP32, name="kw")
                nc.tensor.matmul(kw[:], ksb[:], W_sb[0:C8, :],
                                 start=True, stop=True)
                P_sb = sc.tile([C8 + 1, C], BF16)
                nc.vector.tensor_mul(P_sb[:], kw[:], W_sb[:])
                s1ps = ps_kw.tile([1, C], FP32, name="s1ps")
                nc.tensor.matmul(s1ps[:], ones64[:], P_sb[0:C8, :],
                                 start=True, stop=True)
                s1sb = sc.tile([1, C], FP32, name="s1sb")
                nc.vector.tensor_copy(s1sb[:], s1ps[:])
                nc.sync.dma_start(out=sin[0:1, :], in_=s1sb[:])
                nc.gpsimd.dma_start(out=sin[1:2, :], in_=P_sb[C8:C8 + 1, :])
                nc.gpsimd.collective_compute(
                    "AllReduce", ALU.add,
                    replica_groups=REPLICA_PAIRS,
                    ins=[sin[:]],
                    outs=[sout[:]],
                )
                # x0 instance stats on DVE during the collective wait;
                # tile_wait_until keeps the scheduler from hoisting these
                # ahead of the stage1/loop DVE work (their only data dep is
                # the x0h input DMA, which lands early)
                vc = sc.tile([128, 2], FP32)
                rc = sc.tile([128, 2], FP32)
                with tc.tile_wait_until(0.125):
                    for oc in range(2):
                        for mb in range(8):
                            nc.vector.bn_stats(
                                xst[:, oc, mb, :],
                                x0h_sb[oc][:, mb * 512:(mb + 1) * 512])
                    for oc in range(2):
                        nc.vector.bn_aggr(xagg[:, oc, :], xst[:, oc, :, :])
                    nc.vector.tensor_scalar_add(vc[:], xagg[:, :, 1], EPS)
                    nc.vector.reciprocal(rc[:], vc[:])
                # readback reshaped: Sred[p, r, g] = sout[r, g*128+p]
                Sred = sc.tile([128, 2, 4], FP32)
                nc.sync.dma_start(
                    out=Sred[:],
                    in_=sout[:].rearrange("r (g p) -> p r g", p=128))

                # select own half's channels: msel holds invN (own) / 0,
                # so E2 = S1_own/N and mu0 = S2_own/N after mask-add
                e1 = sc.tile([128, 2, 2], FP32)
                nc.vector.tensor_scalar(e1[:], Sred[:, :, 0:2],
                                        ms_sb[:, 0:1], None, ALU.mult)
                e2b = sc.tile([128, 2, 2], FP32)
                nc.vector.tensor_scalar(e2b[:], Sred[:, :, 2:4],
                                        ms_sb[:, 1:2], None, ALU.mult)
                Eall = sc.tile([128, 2, 2], FP32)
                nc.vector.tensor_add(Eall[:], e1[:], e2b[:])
                E2 = Eall[:, 0, :]
                mu0 = Eall[:, 1, :]
                mus = sc.tile([128, 2], FP32)
                nc.vector.tensor_add(mus[:], mu0, Wb_sb[:])
                m2 = sc.tile([128, 2], FP32)
                nc.vector.tensor_mul(m2[:], mu0, mu0)
                vs = sc.tile([128, 2], FP32)
                nc.vector.tensor_sub(vs[:], E2, m2[:])
                nc.vector.tensor_scalar_add(vs[:], vs[:], EPS)
                ratio = sc.tile([128, 2], FP32)
                nc.vector.tensor_mul(ratio[:], vs[:], rc[:])
                # sqrt(x) = x * rsqrt(x); Quake seed + 2 Newton steps
                ish = sc.tile([128, 2], I32)
                nc.vector.tensor_scalar(ish[:], ratio[:].bitcast(I32),
                                        1, None, ALU.arith_shift_right)
                seed = sc.tile([128, 2], I32)
                nc.vector.tensor_scalar(seed[:], ish[:], -1, 1597463007,
                                        ALU.mult, ALU.add)
                h3 = sc.tile([128, 2], FP32)
                nc.vector.tensor_scalar_mul(h3[:], ratio[:], -0.5)
                yy = seed[:].bitcast(FP32)
                for it in range(2):
                    t1 = sc.tile([128, 2], FP32, name=f"t1_{it}")
                    nc.vector.tensor_mul(t1[:], yy, yy)
                    t2 = sc.tile([128, 2], FP32, name=f"t2_{it}")
                    nc.vector.tensor_mul(t2[:], t1[:], h3[:])
                    t3 = sc.tile([128, 2], FP32, name=f"t3_{it}")
                    nc.vector.tensor_scalar_add(t3[:], t2[:], 1.5)
                    t4 = sc.tile([128, 2], FP32, name=f"t4_{it}")
                    nc.vector.tensor_mul(t4[:], t3[:], yy)
                    yy = t4[:]
                rr = sc.tile([128, 2], FP32)
                nc.vector.tensor_mul(rr[:], ratio[:], yy)
                rmc = sc.tile([128, 2], FP32)
                nc.vector.tensor_mul(rmc[:], rr[:], xagg[:, :, 0])
                tt = sc.tile([128, 2], FP32)
                nc.vector.tensor_sub(tt[:], mus[:], rmc[:])

                # ---- out = r * x0 + t, split across engines/queues ----
                with tc.tile_pool(name="outp", bufs=6) as outp:
                    deng = [nc.sync, nc.gpsimd, nc.scalar]
                    kinds = [1, 0, 1, 2, 1, 0, 1, 2]
                    for ocn in range(2):
                        for mb in range(4):
                            cols = slice(mb * 1024, (mb + 1) * 1024)
                            ot = outp.tile([128, 1024], FP16, name="ot",
                                           tag="ot")
                            kind = kinds[ocn * 4 + mb]
                            if kind == 0:
                                nc.gpsimd.tensor_scalar(
                                    ot[:], x0h_sb[ocn][:, cols],
                                    rr[:, ocn:ocn + 1], tt[:, ocn:ocn + 1],
                                    ALU.mult, ALU.add)
                            elif kind == 1:
                                nc.vector.tensor_scalar(
                                    ot[:], x0h_sb[ocn][:, cols],
                                    rr[:, ocn:ocn + 1], tt[:, ocn:ocn + 1],
                                    ALU.mult, ALU.add)
                            else:
                                nc.scalar.activation(
                                    ot[:], x0h_sb[ocn][:, cols],
                                    ACTF.Identity,
                                    bias=tt[:, ocn:ocn + 1],
                                    scale=rr[:, ocn:ocn + 1])
                            deng[(ocn * 4 + mb) % 3].dma_start(
                                out=out[ocn * 128:(ocn + 1) * 128, cols],
                                in_=ot[:])

    _split_excess_waits(nc)
    return nc


_NC_CACHE = None


def _get_nc():
    global _NC_CACHE
    if _NC_CACHE is None:
        _NC_CACHE = build_nc()
    return _NC_CACHE


def _roll32(xf):
    """Roll each 64-token group by 32: token 64c+a -> 64c+((a+32)%64).
    Output position p holds input token 64*(p//64) + (p%64+32)%64."""
    v = xf.reshape(xf.shape[0], N // 64, 2, 32)
    return np.ascontiguousarray(
        np.concatenate([v[:, :, 1, :], v[:, :, 0, :]], axis=2).reshape(
            xf.shape[0], N))


def make_in_maps(x0, x1, g_w, g_b, theta_w, theta_b, phi_w, phi_b, W_w, W_b):
    f8 = mybir.dt.np(FP8)
    x0f = np.asarray(x0, np.float32).reshape(B, C, N)
    x1f = np.asarray(x1, np.float32).reshape(B, C, N)
    tw = np.asarray(theta_w, np.float32).T * WS       # [C, C8]
    pw = np.asarray(phi_w, np.float32).T * WS
    tpw1 = np.ascontiguousarray(np.concatenate([tw, pw], axis=1)).astype(f8)
    tpw2 = np.ascontiguousarray(np.concatenate([pw, tw], axis=1)).astype(f8)
    tb = np.asarray(theta_b, np.float32)
    pb = np.asarray(phi_b, np.float32)
    tpb1 = np.ascontiguousarray(np.concatenate([tb, pb])[:, None])
    tpb2 = np.ascontiguousarray(np.concatenate([pb, tb])[:, None])
    gw8 = np.ascontiguousarray(np.asarray(g_w, np.float32).T * WS).astype(f8)
    W_wf = np.asarray(W_w, np.float32)                # [C, C8]
    W_bf = np.asarray(W_b, np.float32)
    id65 = np.eye(C8 + 1, dtype=np.float32)
    id128 = np.eye(128, dtype=np.float32)
    gbc = np.ascontiguousarray(np.asarray(g_b, np.float32)[:, None])

    in_maps = []
    for core in range(8):
        b, half = core // 2, core % 2
        Wext = np.concatenate(
            [W_wf.T, np.ones((1, C), np.float32)], axis=0)  # [65, 512] global
        msel = np.zeros((128, 2), np.float32)
        msel[:, half] = 1.0 / N
        x1b = x1f[b] if half == 0 else _roll32(x1f[b])
        x0b = x0f[b] if half == 0 else _roll32(x0f[b])
        in_maps.append({
            "x1_8": x1b.astype(f8),
            "x0_8": x0b.astype(f8),
            "x0h": np.ascontiguousarray(
                x0f[b][half * OC:(half + 1) * OC]).astype(np.float16),
            "tpw1": tpw1,
            "tpw2": tpw2,
            "tpb1": tpb1,
            "tpb2": tpb2,
            "gw8": gw8,
            "Wext": np.ascontiguousarray(Wext),
            "msel": msel,
            "Wb2": np.ascontiguousarray(
                W_bf[half * OC:(half + 1) * OC].reshape(2, 128).T),
            "gbc": gbc,
            "id65": id65,
            "id128": id128,
        })
    return in_maps


def kernel(x0, x1, g_w, g_b, theta_w, theta_b, phi_w, phi_b, W_w, W_b):
    in_maps = make_in_maps(x0, x1, g_w, g_b, theta_w, theta_b, phi_w, phi_b,
                           W_w, W_b)
    nc = _get_nc()
    res = run_bass_kernel_spmd(nc, in_maps, core_ids=list(range(8)))

    outf = np.empty((B, C, N), dtype=np.float32)
    for core in range(8):
        b, half = core // 2, core % 2
        o = np.asarray(res.results[core]["out"], dtype=np.float32)
        outf[b, half * OC:(half + 1) * OC] = o
    return outf.reshape(B, C, H, W)


# revision 33
# speedup vs baseline: 1.0053x; 1.0053x over previous
"""Trainium2 Bass kernel for nn_CrossAttentionBlock (B=4, C=512, H=W=64).

Core = (batch b, query-half h). Queries are split by (token mod 64):
half 0 owns tokens with n%64 in [0,32), half 1 owns [32,64). With the
torch-.view reinterpretation [B,N,C8]->[B,C8,H,W], viewed channel c maps
to y rows [64c, 64c+64); splitting on n%64 makes the W_y per-channel
instance stats a SUM of per-core Gram matrices:
  K[c,c'] = sum_{a,b} y[64c+a, b] y[64c'+a, b]   (a = n%64 within half)
so the only collective is a pairwise AllReduce of K_ext=[K|m] (64x65
fp32, ~17KB) instead of AllGather-ing y (256KB) and recomputing W_y.
  var_s(Cout) = w^T K w / N - (w^T m / N)^2,  mu_s = w^T m / N + W_b
out = r*x0 + t with r = sqrt((var_s+eps)/(var_c+eps)), t = mu_s - r*mu_c.

Per core:
  stage1: theta|phi = conv1x1(x1) with fp8 weights (x16 prescale), fp8 x1.
          Blocks 4-7 use a row-swapped stationary so theta lands on PSUM
          partitions 64-127 -> theta key-chunks 16-31 live at SBUF
          partitions 64-127 for true PE row-tiling. phi kept only for own
          (strided) queries, compacted; duplicated to both partition
          halves via SBUF-SBUF DMA.
  gT:     g^T token-major via x0(fp8)-chunk-stationary matmuls (FWL).
  main:   per key-chunk-pair p: f = theta^T phi for chunks p and p+16
          CONCURRENTLY (row tiles at partitions 0-63 / 64-127); exp on
          ACT (tile A) and Schraudolph-int16 on DVE (tile B); y^T
          accumulated in PSUM over 32 chunks with a ones column giving
          the softmax denominator. 1-pair lookahead pipeline.
  stats:  transpose y^T chunks -> normalize per query (+g_b) -> transpose
          back -> K via 32 strided matmuls -> AllReduce(add) 17KB ->
          KW matmul + reduce -> per-channel r,t -> out = r*x0h + t.
"""
import numpy as np
from contextlib import ExitStack

import concourse.bass as bass
import concourse.tile as tile
from concourse import mybir
from concourse.bass_utils import run_bass_kernel_spmd

FP32 = mybir.dt.float32
BF16 = mybir.dt.bfloat16
FP16 = mybir.dt.float16
FP8 = mybir.dt.float8e4
I16 = mybir.dt.int16
I32 = mybir.dt.int32
ALU = mybir.AluOpType
ACTF = mybir.ActivationFunctionType

B, C, H, W = 4, 512, 64, 64
N = H * W          # 4096 tokens
C8 = C // 8        # 64 inner channels
NQ = N // 2        # 2048 own queries per core
OC = C // 2        # 256 output channels per core
EPS = 1e-5
WS = 16.0          # fp8 weight prescale
IWS = 1.0 / WS

# Schraudolph exp in the bf16 domain: exp(x) ~= bitcast_bf16(int16(A*x+B))
EXPA = float((1 << 7) / np.log(2.0))
EXPB = float(127 * (1 << 7)) - 5.35

REPLICA_PAIRS = [[0, 1], [2, 3], [4, 5], [6, 7]]


def _split_excess_waits(nc, max_waits=1, drain_max=1):
    """walrus rejects instructions carrying more than ~2 sync waits; move
    extras to preceding NoOps on the same engine."""
    for blk in nc.main_func.blocks:
        insts = blk.instructions
        k = 0
        while k < len(insts):
            inst = insts[k]
            si = inst.sync_info
            cap = drain_max if inst.opcode == "Drain" else max_waits
            if si is not None and si.on_wait and len(si.on_wait) > cap:
                waits = list(si.on_wait)
                keep = waits[-cap:]
                extra = waits[:-cap]
                pos = k
                for j in range(0, len(extra), cap):
                    nop = mybir.InstNoOp(name=f"{inst.name}-wsplit{j}", ins=[], outs=[])
                    nop.engine = inst.engine
                    nop.sync_info = mybir.SyncInfo(
                        on_wait=extra[j : j + cap], on_update=[]
                    )
                    insts.insert(pos, nop)
                    pos += 1
                    k += 1
                inst.sync_info = mybir.SyncInfo(on_wait=keep, on_update=list(si.on_update))
            k += 1


def build_nc():
    """One SPMD program for all cores. Each core owns queries with
    n%64 in [0,32) of ITS (possibly group-rolled) token order; odd cores
    get x1/x0_8 rolled by 32 within each 64-token group on the host, so
    the kernel's strided-phi APs are core-independent."""
    h = 0
    nc = bass.Bass()

    x1_8 = nc.dram_tensor("x1_8", [C, N], FP8, kind="ExternalInput")
    x0_8 = nc.dram_tensor("x0_8", [C, N], FP8, kind="ExternalInput")
    x0h = nc.dram_tensor("x0h", [OC, N], FP16, kind="ExternalInput")
    tpw1 = nc.dram_tensor("tpw1", [C, 128], FP8, kind="ExternalInput")
    tpw2 = nc.dram_tensor("tpw2", [C, 128], FP8, kind="ExternalInput")
    tpb1 = nc.dram_tensor("tpb1", [128, 1], FP32, kind="ExternalInput")
    tpb2 = nc.dram_tensor("tpb2", [128, 1], FP32, kind="ExternalInput")
    gw8 = nc.dram_tensor("gw8", [C, C8], FP8, kind="ExternalInput")
    Wext = nc.dram_tensor("Wext", [C8 + 1, C], FP32, kind="ExternalInput")
    msel = nc.dram_tensor("msel", [128, 2], FP32, kind="ExternalInput")
    Wb2 = nc.dram_tensor("Wb2", [128, 2], FP32, kind="ExternalInput")
    gbc = nc.dram_tensor("gbc", [C8, 1], FP32, kind="ExternalInput")
    id65 = nc.dram_tensor("id65", [C8 + 1, C8 + 1], FP32, kind="ExternalInput")
    id128 = nc.dram_tensor("id128", [128, 128], FP32, kind="ExternalInput")
    out = nc.dram_tensor("out", [OC, N], FP16, kind="ExternalOutput")

    sin = nc.dram_tensor("sin", [2, 512], FP32)
    sout = nc.dram_tensor("sout", [2, 512], FP32)
    ccw_in = nc.dram_tensor("cc_warm_in", [1, 16], FP32)
    ccw_out = nc.dram_tensor("cc_warm_out", [1, 16], FP32)
    ccw2_in = nc.dram_tensor("ccw2_in", [1, 16], FP32)
    ccw2_out = nc.dram_tensor("ccw2_out", [1, 16], FP32)
    ccw3_in = nc.dram_tensor("ccw3_in", [1, 16], BF16)
    ccw3_out = nc.dram_tensor("ccw3_out", [1, 16], BF16)
    ccw4_in = nc.dram_tensor("ccw4_in", [1, 16], FP32)
    ccw4_out = nc.dram_tensor("ccw4_out", [1, 16], FP32)

    with tile.TileContext(nc) as tc, ExitStack() as ctx:
        wpool = ctx.enter_context(tc.tile_pool(name="weights", bufs=1))
        big = ctx.enter_context(tc.tile_pool(name="big", bufs=1))

        # ---- weight tiles ----
        tpw1_sb = wpool.tile([128, 4, 128], FP8)
        tpw2_sb = wpool.tile([128, 4, 128], FP8)
        gw_sb = wpool.tile([128, 4, C8], FP8)
        tpb1_sb = wpool.tile([128, 1], FP32)
        tpb2_sb = wpool.tile([128, 1], FP32)
        W_sb = wpool.tile([C8 + 1, C], FP32)
        ms_sb = wpool.tile([128, 2], FP32)
        Wb_sb = wpool.tile([128, 2], FP32)
        gb_sb = wpool.tile([C8, 1], FP32)
        id65_sb = wpool.tile([C8 + 1, C8 + 1], FP32)
        id128_sb = wpool.tile([128, 128], FP32)
        ones64 = wpool.tile([C8, 1], BF16)

        # ---- persistent big tensors (per-chunk tiles => subtile deps) ----
        x1c = [big.tile([128, N], FP8, name=f"x1c{c}") for c in range(4)]
        x0c = [big.tile([128, N], FP8, name=f"x0c{c}") for c in range(4)]
        x0h_sb = [big.tile([128, N], FP16, name=f"x0h{o}") for o in range(2)]

        # critical weights first (small), on all 3 queues
        for c in range(4):
            eng3 = [nc.sync, nc.scalar, nc.gpsimd][c % 3]
            eng3.dma_start(out=tpw1_sb[:, c, :], in_=tpw1[c * 128:(c + 1) * 128, :])
        nc.sync.dma_start(out=tpb1_sb[:], in_=tpb1[:])
        nc.scalar.dma_start(out=tpb2_sb[:], in_=tpb2[:])
        thAB = big.tile([128, 16, 128], BF16)   # theta; rows 0-63 chunks 0-15,
                                                # rows 64-127 chunks 16-31
        ph2 = big.tile([128, NQ], BF16)         # own-query phi, both halves
        g_ext = big.tile([128, 32, C8 + 2], BF16)  # gT chunks + ones col
        yraw = big.tile([C8 + 1, NQ], FP32)     # yT_ext (pre-normalization)
        ynx = big.tile([128, 16, C8], FP32)     # transposed normalized y
        yn = big.tile([C8, NQ + C8], BF16)      # channel-major y + ones cols

        # warm the exp table + CC stack early (before gpsimd's DMA pacing)
        warm = wpool.tile([128, 1], FP32)
        nc.scalar.activation(warm[:], tpb1_sb[:], ACTF.Exp)
        nc.gpsimd.memset(g_ext[:, :, C8:C8 + 1], 1.0)
        nc.gpsimd.memset(yn[:, NQ:NQ + 32], 1.0)
        nc.gpsimd.memset(ones64[:], 1.0)
        nc.gpsimd.collective_compute(
            "AllReduce", ALU.add,
            replica_groups=REPLICA_PAIRS,
            ins=[ccw_in[:]],
            outs=[ccw_out[:]],
        )

        # ---- input DMAs: x1 on all 3 queues (scalar's triggers drain
        # before ACT's first exp), everything else on sync+gpsimd ----
        eng3 = [nc.sync, nc.scalar, nc.gpsimd]
        k = 0
        for q in range(4):
            cols = slice(q * 1024, (q + 1) * 1024)
            for c in range(4):
                eng3[k % 3].dma_start(out=x1c[c][:, cols],
                                      in_=x1_8[c * 128:(c + 1) * 128, cols])
                k += 1
            if q == 0:
                # tpw2 needed from stage1 block 4 on
                for c in range(4):
                    eng3[(k + c) % 3].dma_start(
                        out=tpw2_sb[:, c, :], in_=tpw2[c * 128:(c + 1) * 128, :])
        eng2 = [nc.sync, nc.gpsimd]
        for c in range(4):
            eng2[c % 2].dma_start(out=gw_sb[:, c, :],
                                  in_=gw8[c * 128:(c + 1) * 128, :])
        k = 0
        for q in range(4):
            cols = slice(q * 1024, (q + 1) * 1024)
            for c in range(4):
                eng2[k % 2].dma_start(out=x0c[c][:, cols],
                                      in_=x0_8[c * 128:(c + 1) * 128, cols])
                k += 1
        for oc in range(2):
            eng2[oc % 2].dma_start(out=x0h_sb[oc][:],
                                   in_=x0h[oc * 128:(oc + 1) * 128, :])
        # late-needed small tensors at the queue tails
        nc.sync.dma_start(out=id65_sb[:], in_=id65[:])
        nc.gpsimd.dma_start(out=id128_sb[:], in_=id128[:])
        nc.gpsimd.dma_start(out=W_sb[:], in_=Wext[:])
        nc.gpsimd.dma_start(out=ms_sb[:], in_=msel[:])
        nc.sync.dma_start(out=Wb_sb[:], in_=Wb2[:])
        nc.sync.dma_start(out=gb_sb[:], in_=gbc[:])

        # ---- stage 1: x1 -> theta/phi (fp8 weights, x16 prescale) ----
        with tc.tile_pool(name="ps_tp", bufs=2, space="PSUM") as ps_tp:
            for b in range(8):
                cols = slice(b * 512, (b + 1) * 512)
                tpw_sb = tpw1_sb if b < 4 else tpw2_sb
                ptp = ps_tp.tile([128, 512], FP32, name="ptp")
                for c in range(4):
                    nc.tensor.matmul(ptp[:], tpw_sb[:, c, :], x1c[c][:, cols],
                                     start=(c == 0), stop=(c == 3))
                trows = slice(0, 64) if b < 4 else slice(64, 128)
                prows = slice(64, 128) if b < 4 else slice(0, 64)
                tpb_sb = tpb1_sb if b < 4 else tpb2_sb
                ch = (b % 4) * 4
                # theta (full block) on DVE: (psum * 1/16) + bias
                nc.vector.tensor_scalar(
                    thAB[trows, ch:ch + 4, :].rearrange("p a b -> p (a b)"),
                    ptp[trows, :], IWS, tpb_sb[trows, :],
                    ALU.mult, ALU.add)
                # phi (own strided queries, compacted) on DVE
                nc.vector.tensor_scalar(
                    ph2[prows, b * 256:(b + 1) * 256].rearrange(
                        "p (g k) -> p g k", k=32),
                    ptp[prows, :].rearrange("p (g k) -> p g k", k=64)[
                        :, :, 32 * h:32 * h + 32],
                    IWS, tpb_sb[prows, :], ALU.mult, ALU.add)

        # phi lives at rows 64-127 for blocks 0-3, rows 0-63 for blocks 4-7;
        # duplicate each half to the other partition range (SBUF-SBUF DMA)
        nc.scalar.dma_start(out=ph2[0:64, 0:1024], in_=ph2[64:128, 0:1024])
        nc.scalar.dma_start(out=ph2[64:128, 1024:2048], in_=ph2[0:64, 1024:2048])

        # ---- gT: x0 chunks stationary (fp8, FWL), gw moving ----
        with tc.tile_pool(name="ps_g", bufs=3, space="PSUM") as ps_g:
            for mj in range(16):
                pg = ps_g.tile([128, 2, C8], FP32, name="pg")
                for half in range(2):
                    mi = mj * 2 + half
                    for c in range(4):
                        nc.tensor.matmul(pg[:, half, :],
                                         x0c[c][:, mi * 128:(mi + 1) * 128],
                                         gw_sb[:, c, :],
                                         start=(c == 0), stop=(c == 3))
                nc.vector.tensor_scalar(
                    g_ext[:, mj * 2:mj * 2 + 2, 0:C8], pg[:], IWS, None,
                    ALU.mult)

        # ---- main attention loop ----
        stat = ctx.enter_context(tc.tile_pool(name="stats", bufs=1))
        xst = stat.tile([128, 2, 8, 6], FP32)
        xagg = stat.tile([128, 2, 2], FP32)

        def emit_fwd_T(j, ps_t, ystage):
            ptile = ps_t.tile([128, C8 + 1], FP32, name="ptile", tag="pt")
            nc.tensor.transpose(ptile[:], yraw[:, j * 128:(j + 1) * 128],
                                id65_sb[:])
            rec = ystage.tile([128, 1], FP32, name="rec", tag="rec")
            nc.vector.reciprocal(rec[:], ptile[:, C8:C8 + 1])
            nc.vector.tensor_scalar(ynx[:, j, :], ptile[:, 0:C8], rec[:],
                                    None, ALU.mult)

        def emit_back_T(j, ps_t2):
            pt2 = ps_t2.tile([C8, 128], FP32, name="pt2", tag="pt2")
            nc.tensor.transpose(pt2[:], ynx[:, j, :], id128_sb[:])
            nc.vector.tensor_scalar(
                yn[:, j * 128:(j + 1) * 128], pt2[:], gb_sb[:], None,
                ALU.add)

        with tc.tile_pool(name="ps_fA", bufs=3, space="PSUM") as ps_fA, \
             tc.tile_pool(name="ps_fB", bufs=3, space="PSUM") as ps_fB, \
             tc.tile_pool(name="ps_y", bufs=1, space="PSUM") as ps_y, \
             tc.tile_pool(name="ptA", bufs=4) as ptA_pool, \
             tc.tile_pool(name="ptB", bufs=4) as ptB_pool:
            for pss in range(2):
                py = ps_y.tile([C8 + 1, 2, 512], FP32, name="py")

                def emit_y(args):
                    p, pas, pbs = args
                    for s in range(2):
                        nc.tensor.matmul(py[:, s, :], g_ext[:, p, 0:C8 + 1],
                                         pas[s][:], start=(p == 0), stop=False)
                    for s in range(2):
                        nc.tensor.matmul(py[:, s, :], g_ext[:, p + 16, 0:C8 + 1],
                                         pbs[s][:],
                                         start=False, stop=(p == 15))

                prev = None
                for p in range(16):
                    fas, fbs, pas, pbs = [], [], [], []
                    for s in range(2):
                        qs = slice(pss * 1024 + s * 512,
                                   pss * 1024 + (s + 1) * 512)
                        fa = ps_fA.tile([128, 512], FP32, name="fa", tag="fa")
                        nc.tensor.matmul(fa[:], thAB[0:64, p, :], ph2[0:64, qs],
                                         start=True, stop=True)
                        fas.append(fa)
                    for s in range(2):
                        qs = slice(pss * 1024 + s * 512,
                                   pss * 1024 + (s + 1) * 512)
                        fb = ps_fB.tile([128, 512], FP32, name="fb", tag="fb")
                        nc.tensor.matmul(fb[:], thAB[64:128, p, :],
                                         ph2[64:128, qs],
                                         start=True, stop=True)
                        fbs.append(fb)
                    for s in range(2):
                        pa = ptA_pool.tile([128, 512], BF16, name="pa", tag="pa")
                        nc.scalar.activation(pa[:], fas[s][:], ACTF.Exp)
                        pas.append(pa)
                        pb = ptB_pool.tile([128, 512], BF16, name="pb", tag="pb")
                        if s == 0:
                            nc.vector.tensor_scalar(pb[:].bitcast(I16),
                                                    fbs[s][:], EXPA, EXPB,
                                                    ALU.mult, ALU.add)
                        else:
                            # rebalance: DVE half, ACT half
                            nc.vector.tensor_scalar(
                                pb[:, 0:256].bitcast(I16), fbs[s][:, 0:256],
                                EXPA, EXPB, ALU.mult, ALU.add)
                            nc.scalar.activation(pb[:, 256:512],
                                                 fbs[s][:, 256:512], ACTF.Exp)
                        pbs.append(pb)
                    if prev is not None:
                        emit_y(prev)
                    prev = (p, pas, pbs)
                emit_y(prev)

                nc.vector.tensor_copy(
                    yraw[:, pss * 1024:(pss + 1) * 1024],
                    py[:].rearrange("p a b -> p (a b)"))
                if pss == 0:
                    nc.scalar.dma_start(out=ccw2_in[:],
                                        in_=yraw[C8:C8 + 1, 0:16])
                    nc.gpsimd.collective_compute(
                        "AllReduce", ALU.add,
                        replica_groups=REPLICA_PAIRS,
                        ins=[ccw2_in[:]], outs=[ccw2_out[:]])

        # ---- transpose yT chunks, normalize, transpose back ----
        with tc.tile_pool(name="ps_t", bufs=3, space="PSUM") as ps_t, \
             tc.tile_pool(name="ps_t2", bufs=3, space="PSUM") as ps_t2, \
             tc.tile_pool(name="ystage", bufs=4) as ystage:
            for j in range(16):
                emit_fwd_T(j, ps_t, ystage)
                emit_back_T(j, ps_t2)

        # ---- K = sum_a yn_a yn_a^T (+ mean col via ones), AllReduce ----
        yn_r = yn[:].rearrange("p (c a) -> p c a", a=32)
        with tc.tile_pool(name="ps_k", bufs=1, space="PSUM") as ps_k, \
             tc.tile_pool(name="kst", bufs=1) as kst:
            kps = ps_k.tile([C8, C8 + 1], FP32, name="kps")
            for a in range(32):
                nc.tensor.matmul(
                    kps[:],
                    yn_r[:, 0:C8, a:a + 1].rearrange("p c o -> p (c o)"),
                    yn_r[:, 0:C8 + 1, a:a + 1].rearrange("p c o -> p (c o)"),
                    start=(a == 0), stop=(a == 31))
            ksb = kst.tile([C8, C8 + 1], FP32, name="ksb")
            nc.vector.tensor_copy(ksb[:], kps[:])

            # ---- per-channel S1 = w^T K_own w, S2 = m_own^T w (linear in
            # K => AllReduce the [128,4] scalars instead of K itself) ----
            with tc.tile_pool(name="ps_kw", bufs=1, space="PSUM") as ps_kw, \
                 tc.tile_pool(name="sc", bufs=1) as sc:
                # partial S for ALL 512 out channels in GLOBAL order so
                # the pairwise AllReduce adds matching quadratic forms;
                # S1 = column sums of P via ones^T @ P, S2 = P row 64
                kw = ps_kw.tile([C8 + 1, C], FP32, name="kw")
                nc.tensor.matmul(kw[:], ksb[:], W_sb[0:C8, :],
                                 start=True, stop=True)
                P_sb = sc.tile([C8 + 1, C], BF16)
                nc.vector.tensor_mul(P_sb[:], kw[:], W_sb[:])
                s1ps = ps_kw.tile([1, C], FP32, name="s1ps")
                nc.tensor.matmul(s1ps[:], ones64[:], P_sb[0:C8, :],
                                 start=True, stop=True)
                s1sb = sc.tile([1, C], FP32, name="s1sb")
                nc.vector.tensor_copy(s1sb[:], s1ps[:])
                nc.sync.dma_start(out=sin[0:1, :], in_=s1sb[:])
                nc.gpsimd.dma_start(out=sin[1:2, :], in_=P_sb[C8:C8 + 1, :])
                nc.gpsimd.collective_compute(
                    "AllReduce", ALU.add,
                    replica_groups=REPLICA_PAIRS,
                    ins=[sin[:]],
                    outs=[sout[:]],
                )
                # x0 instance stats on DVE during the collective wait;
                # tile_wait_until keeps the scheduler from hoisting these
                # ahead of the stage1/loop DVE work (their only data dep is
                # the x0h input DMA, which lands early)
                vc = sc.tile([128, 2], FP32)
                rc = sc.tile([128, 2], FP32)
                with tc.tile_wait_until(0.125):
                    for oc in range(2):
                        for mb in range(8):
                            nc.vector.bn_stats(
                                xst[:, oc, mb, :],
                                x0h_sb[oc][:, mb * 512:(mb + 1) * 512])
                    for oc in range(2):
                        nc.vector.bn_aggr(xagg[:, oc, :], xst[:, oc, :, :])
                    nc.vector.tensor_scalar_add(vc[:], xagg[:, :, 1], EPS)
                    nc.vector.reciprocal(rc[:], vc[:])
                    # pre-switch the ACT table Exp->Sqrt while CC runs
                    warm2 = sc.tile([128, 1], FP32, name="warm2")
                    nc.scalar.activation(warm2[:], tpb1_sb[:], ACTF.Sqrt)
                # readback reshaped: Sred[p, r, g] = sout[r, g*128+p]
                Sred = sc.tile([128, 2, 4], FP32)
                nc.sync.dma_start(
                    out=Sred[:],
                    in_=sout[:].rearrange("r (g p) -> p r g", p=128))

                # select own half's channels: msel holds invN (own) / 0,
                # so E2 = S1_own/N and mu0 = S2_own/N after mask-add
                e1 = sc.tile([128, 2, 2], FP32)
                nc.vector.tensor_scalar(e1[:], Sred[:, :, 0:2],
                                        ms_sb[:, 0:1], None, ALU.mult)
                e2b = sc.tile([128, 2, 2], FP32)
                nc.vector.tensor_scalar(e2b[:], Sred[:, :, 2:4],
                                        ms_sb[:, 1:2], None, ALU.mult)
                Eall = sc.tile([128, 2, 2], FP32)
                nc.vector.tensor_add(Eall[:], e1[:], e2b[:])
                E2 = Eall[:, 0, :]
                mu0 = Eall[:, 1, :]
                mus = sc.tile([128, 2], FP32)
                nc.vector.tensor_add(mus[:], mu0, Wb_sb[:])
                m2 = sc.tile([128, 2], FP32)
                nc.vector.tensor_mul(m2[:], mu0, mu0)
                vs = sc.tile([128, 2], FP32)
                nc.vector.tensor_sub(vs[:], E2, m2[:])
                nc.vector.tensor_scalar_add(vs[:], vs[:], EPS)
                ratio = sc.tile([128, 2], FP32)
                nc.vector.tensor_mul(ratio[:], vs[:], rc[:])
                # r = sqrt(ratio) on ACT (table pre-switched)
                rr = sc.tile([128, 2], FP32)
                nc.scalar.activation(rr[:], ratio[:], ACTF.Sqrt)
                rmc = sc.tile([128, 2], FP32)
                nc.vector.tensor_mul(rmc[:], rr[:], xagg[:, :, 0])
                tt = sc.tile([128, 2], FP32)
                nc.vector.tensor_sub(tt[:], mus[:], rmc[:])

                # ---- out = r * x0 + t, split across engines/queues ----
                with tc.tile_pool(name="outp", bufs=6) as outp:
                    deng = [nc.sync, nc.gpsimd, nc.scalar]
                    kinds = [1, 0, 1, 2, 1, 0, 1, 2]
                    for ocn in range(2):
                        for mb in range(4):
                            cols = slice(mb * 1024, (mb + 1) * 1024)
                            ot = outp.tile([128, 1024], FP16, name="ot",
                                           tag="ot")
                            kind = kinds[ocn * 4 + mb]
                            if kind == 0:
                                nc.gpsimd.tensor_scalar(
                                    ot[:], x0h_sb[ocn][:, cols],
                                    rr[:, ocn:ocn + 1], tt[:, ocn:ocn + 1],
                                    ALU.mult, ALU.add)
                            elif kind == 1:
                                nc.vector.tensor_scalar(
                                    ot[:], x0h_sb[ocn][:, cols],
                                    rr[:, ocn:ocn + 1], tt[:, ocn:ocn + 1],
                                    ALU.mult, ALU.add)
                            else:
                                nc.scalar.activation(
                                    ot[:], x0h_sb[ocn][:, cols],
                                    ACTF.Identity,
                                    bias=tt[:, ocn:ocn + 1],
                                    scale=rr[:, ocn:ocn + 1])
                            deng[(ocn * 4 + mb) % 3].dma_start(
                                out=out[ocn * 128:(ocn + 1) * 128, cols],
                                in_=ot[:])

    _split_excess_waits(nc)
    return nc


_NC_CACHE = None


def _get_nc():
    global _NC_CACHE
    if _NC_CACHE is None:
        _NC_CACHE = build_nc()
    return _NC_CACHE


def _roll32(xf):
    """Roll each 64-token group by 32: token 64c+a -> 64c+((a+32)%64).
    Output position p holds input token 64*(p//64) + (p%64+32)%64."""
    v = xf.reshape(xf.shape[0], N // 64, 2, 32)
    return np.ascontiguousarray(
        np.concatenate([v[:, :, 1, :], v[:, :, 0, :]], axis=2).reshape(
            xf.shape[0], N))


def make_in_maps(x0, x1, g_w, g_b, theta_w, theta_b, phi_w, phi_b, W_w, W_b):
    f8 = mybir.dt.np(FP8)
    x0f = np.asarray(x0, np.float32).reshape(B, C, N)
    x1f = np.asarray(x1, np.float32).reshape(B, C, N)
    tw = np.asarray(theta_w, np.float32).T * WS       # [C, C8]
    pw = np.asarray(phi_w, np.float32).T * WS
    tpw1 = np.ascontiguousarray(np.concatenate([tw, pw], axis=1)).astype(f8)
    tpw2 = np.ascontiguousarray(np.concatenate([pw, tw], axis=1)).astype(f8)
    tb = np.asarray(theta_b, np.float32)
    pb = np.asarray(phi_b, np.float32)
    tpb1 = np.ascontiguousarray(np.concatenate([tb, pb])[:, None])
    tpb2 = np.ascontiguousarray(np.concatenate([pb, tb])[:, None])
    gw8 = np.ascontiguousarray(np.asarray(g_w, np.float32).T * WS).astype(f8)
    W_wf = np.asarray(W_w, np.float32)                # [C, C8]
    W_bf = np.asarray(W_b, np.float32)
    id65 = np.eye(C8 + 1, dtype=np.float32)
    id128 = np.eye(128, dtype=np.float32)
    gbc = np.ascontiguousarray(np.asarray(g_b, np.float32)[:, None])

    in_maps = []
    for core in range(8):
        b, half = core // 2, core % 2
        Wext = np.concatenate(
            [W_wf.T, np.ones((1, C), np.float32)], axis=0)  # [65, 512] global
        msel = np.zeros((128, 2), np.float32)
        msel[:, half] = 1.0 / N
        x1b = x1f[b] if half == 0 else _roll32(x1f[b])
        x0b = x0f[b] if half == 0 else _roll32(x0f[b])
        in_maps.append({
            "x1_8": x1b.astype(f8),
            "x0_8": x0b.astype(f8),
            "x0h": np.ascontiguousarray(
                x0f[b][half * OC:(half + 1) * OC]).astype(np.float16),
            "tpw1": tpw1,
            "tpw2": tpw2,
            "tpb1": tpb1,
            "tpb2": tpb2,
            "gw8": gw8,
            "Wext": np.ascontiguousarray(Wext),
            "msel": msel,
            "Wb2": np.ascontiguousarray(
                W_bf[half * OC:(half + 1) * OC].reshape(2, 128).T),
            "gbc": gbc,
            "id65": id65,
            "id128": id128,
        })
    return in_maps


def kernel(x0, x1, g_w, g_b, theta_w, theta_b, phi_w, phi_b, W_w, W_b):
    in_maps = make_in_maps(x0, x1, g_w, g_b, theta_w, theta_b, phi_w, phi_b,
                           W_w, W_b)
    nc = _get_nc()
    res = run_bass_kernel_spmd(nc, in_maps, core_ids=list(range(8)))

    outf = np.empty((B, C, N), dtype=np.float32)
    for core in range(8):
        b, half = core // 2, core % 2
        o = np.asarray(res.results[core]["out"], dtype=np.float32)
        outf[b, half * OC:(half + 1) * OC] = o
    return outf.reshape(B, C, H, W)


# revision 34
# speedup vs baseline: 1.0544x; 1.0488x over previous
"""Trainium2 Bass kernel for nn_CrossAttentionBlock (B=4, C=512, H=W=64).

Core = (batch b, query-half h). Queries are split by (token mod 64):
half 0 owns tokens with n%64 in [0,32), half 1 owns [32,64). With the
torch-.view reinterpretation [B,N,C8]->[B,C8,H,W], viewed channel c maps
to y rows [64c, 64c+64); splitting on n%64 makes the W_y per-channel
instance stats a SUM of per-core Gram matrices:
  K[c,c'] = sum_{a,b} y[64c+a, b] y[64c'+a, b]   (a = n%64 within half)
so the only collective is a pairwise AllReduce of K_ext=[K|m] (64x65
fp32, ~17KB) instead of AllGather-ing y (256KB) and recomputing W_y.
  var_s(Cout) = w^T K w / N - (w^T m / N)^2,  mu_s = w^T m / N + W_b
out = r*x0 + t with r = sqrt((var_s+eps)/(var_c+eps)), t = mu_s - r*mu_c.

Per core:
  stage1: theta|phi = conv1x1(x1) with fp8 weights (x16 prescale), fp8 x1.
          Blocks 4-7 use a row-swapped stationary so theta lands on PSUM
          partitions 64-127 -> theta key-chunks 16-31 live at SBUF
          partitions 64-127 for true PE row-tiling. phi kept only for own
          (strided) queries, compacted; duplicated to both partition
          halves via SBUF-SBUF DMA.
  gT:     g^T token-major via x0(fp8)-chunk-stationary matmuls (FWL).
  main:   per key-chunk-pair p: f = theta^T phi for chunks p and p+16
          CONCURRENTLY (row tiles at partitions 0-63 / 64-127); exp on
          ACT (tile A) and Schraudolph-int16 on DVE (tile B); y^T
          accumulated in PSUM over 32 chunks with a ones column giving
          the softmax denominator. 1-pair lookahead pipeline.
  stats:  transpose y^T chunks -> normalize per query (+g_b) -> transpose
          back -> K via 32 strided matmuls -> AllReduce(add) 17KB ->
          KW matmul + reduce -> per-channel r,t -> out = r*x0h + t.
"""
import numpy as np
from contextlib import ExitStack

import concourse.bass as bass
import concourse.tile as tile
from concourse import mybir
from concourse.bass_utils import run_bass_kernel_spmd

FP32 = mybir.dt.float32
BF16 = mybir.dt.bfloat16
FP16 = mybir.dt.float16
FP8 = mybir.dt.float8e4
I16 = mybir.dt.int16
I32 = mybir.dt.int32
ALU = mybir.AluOpType
ACTF = mybir.ActivationFunctionType

B, C, H, W = 4, 512, 64, 64
N = H * W          # 4096 tokens
C8 = C // 8        # 64 inner channels
NQ = N // 2        # 2048 own queries per core
OC = C // 2        # 256 output channels per core
EPS = 1e-5
WS = 16.0          # fp8 weight prescale
IWS = 1.0 / WS

# Schraudolph exp in the bf16 domain: exp(x) ~= bitcast_bf16(int16(A*x+B))
EXPA = float((1 << 7) / np.log(2.0))
EXPB = float(127 * (1 << 7)) - 5.35

REPLICA_PAIRS = [[0, 1], [2, 3], [4, 5], [6, 7]]


def _split_excess_waits(nc, max_waits=1, drain_max=1):
    """walrus rejects instructions carrying more than ~2 sync waits; move
    extras to preceding NoOps on the same engine."""
    for blk in nc.main_func.blocks:
        insts = blk.instructions
        k = 0
        while k < len(insts):
            inst = insts[k]
            si = inst.sync_info
            cap = drain_max if inst.opcode == "Drain" else max_waits
            if si is not None and si.on_wait and len(si.on_wait) > cap:
                waits = list(si.on_wait)
                keep = waits[-cap:]
                extra = waits[:-cap]
                pos = k
                for j in range(0, len(extra), cap):
                    nop = mybir.InstNoOp(name=f"{inst.name}-wsplit{j}", ins=[], outs=[])
                    nop.engine = inst.engine
                    nop.sync_info = mybir.SyncInfo(
                        on_wait=extra[j : j + cap], on_update=[]
                    )
                    insts.insert(pos, nop)
                    pos += 1
                    k += 1
                inst.sync_info = mybir.SyncInfo(on_wait=keep, on_update=list(si.on_update))
            k += 1


def build_nc():
    """One SPMD program for all cores. Each core owns queries with
    n%64 in [0,32) of ITS (possibly group-rolled) token order; odd cores
    get x1/x0_8 rolled by 32 within each 64-token group on the host, so
    the kernel's strided-phi APs are core-independent."""
    h = 0
    nc = bass.Bass()

    x1_8 = nc.dram_tensor("x1_8", [C, N], FP8, kind="ExternalInput")
    x0_8 = nc.dram_tensor("x0_8", [C, N], FP8, kind="ExternalInput")
    x0h = nc.dram_tensor("x0h", [OC, N], FP16, kind="ExternalInput")
    tpw1 = nc.dram_tensor("tpw1", [C, 128], FP8, kind="ExternalInput")
    tpw2 = nc.dram_tensor("tpw2", [C, 128], FP8, kind="ExternalInput")
    tpb1 = nc.dram_tensor("tpb1", [128, 1], FP32, kind="ExternalInput")
    tpb2 = nc.dram_tensor("tpb2", [128, 1], FP32, kind="ExternalInput")
    gw8 = nc.dram_tensor("gw8", [C, C8], FP8, kind="ExternalInput")
    Wext = nc.dram_tensor("Wext", [C8 + 1, C], FP32, kind="ExternalInput")
    msel = nc.dram_tensor("msel", [128, 2], FP32, kind="ExternalInput")
    Wb2 = nc.dram_tensor("Wb2", [128, 2], FP32, kind="ExternalInput")
    gbc = nc.dram_tensor("gbc", [C8, 1], FP32, kind="ExternalInput")
    id65 = nc.dram_tensor("id65", [C8 + 1, C8 + 1], FP32, kind="ExternalInput")
    id128 = nc.dram_tensor("id128", [128, 128], FP32, kind="ExternalInput")
    out = nc.dram_tensor("out", [OC, N], FP16, kind="ExternalOutput")

    sin = nc.dram_tensor("sin", [2, 512], FP32)
    sout = nc.dram_tensor("sout", [2, 512], FP32)
    ccw_in = nc.dram_tensor("cc_warm_in", [1, 16], FP32)
    ccw_out = nc.dram_tensor("cc_warm_out", [1, 16], FP32)
    ccw2_in = nc.dram_tensor("ccw2_in", [1, 16], FP32)
    ccw2_out = nc.dram_tensor("ccw2_out", [1, 16], FP32)
    ccw3_in = nc.dram_tensor("ccw3_in", [1, 16], BF16)
    ccw3_out = nc.dram_tensor("ccw3_out", [1, 16], BF16)
    ccw4_in = nc.dram_tensor("ccw4_in", [1, 16], FP32)
    ccw4_out = nc.dram_tensor("ccw4_out", [1, 16], FP32)

    with tile.TileContext(nc) as tc, ExitStack() as ctx:
        wpool = ctx.enter_context(tc.tile_pool(name="weights", bufs=1))
        big = ctx.enter_context(tc.tile_pool(name="big", bufs=1))

        # ---- weight tiles ----
        tpw1_sb = wpool.tile([128, 4, 128], FP8)
        tpw2_sb = wpool.tile([128, 4, 128], FP8)
        gw_sb = wpool.tile([128, 4, C8], FP8)
        tpb1_sb = wpool.tile([128, 1], FP32)
        tpb2_sb = wpool.tile([128, 1], FP32)
        W_sb = wpool.tile([C8 + 1, C], FP32)
        ms_sb = wpool.tile([128, 2], FP32)
        Wb_sb = wpool.tile([128, 2], FP32)
        gb_sb = wpool.tile([C8, 1], FP32)
        id65_sb = wpool.tile([C8 + 1, C8 + 1], FP32)
        id128_sb = wpool.tile([128, 128], FP32)
        ones64 = wpool.tile([C8, 1], BF16)

        # ---- persistent big tensors (per-chunk tiles => subtile deps) ----
        x1c = [big.tile([128, N], FP8, name=f"x1c{c}") for c in range(4)]
        x0c = [big.tile([128, N], FP8, name=f"x0c{c}") for c in range(4)]
        x0h_sb = [big.tile([128, N], FP16, name=f"x0h{o}") for o in range(2)]

        # critical weights first (small), on all 3 queues
        for c in range(4):
            eng3 = [nc.sync, nc.scalar, nc.gpsimd][c % 3]
            eng3.dma_start(out=tpw1_sb[:, c, :], in_=tpw1[c * 128:(c + 1) * 128, :])
        nc.sync.dma_start(out=tpb1_sb[:], in_=tpb1[:])
        nc.scalar.dma_start(out=tpb2_sb[:], in_=tpb2[:])
        thAB = big.tile([128, 16, 128], BF16)   # theta; rows 0-63 chunks 0-15,
                                                # rows 64-127 chunks 16-31
        ph2 = big.tile([128, NQ], BF16)         # own-query phi, both halves
        g_ext = big.tile([128, 32, C8 + 2], BF16)  # gT chunks + ones col
        yraw = big.tile([C8 + 1, NQ], FP32)     # yT_ext (pre-normalization)
        ynx = big.tile([128, 16, C8], FP32)     # transposed normalized y
        yn = big.tile([C8, NQ + C8], BF16)      # channel-major y + ones cols

        # warm the exp table + CC stack early (before gpsimd's DMA pacing)
        warm = wpool.tile([128, 1], FP32)
        nc.scalar.activation(warm[:], tpb1_sb[:], ACTF.Exp)
        nc.gpsimd.memset(g_ext[:, :, C8:C8 + 1], 1.0)
        nc.gpsimd.memset(yn[:, NQ:NQ + 32], 1.0)
        nc.gpsimd.memset(ones64[:], 1.0)
        nc.gpsimd.collective_compute(
            "AllReduce", ALU.add,
            replica_groups=REPLICA_PAIRS,
            ins=[ccw_in[:]],
            outs=[ccw_out[:]],
        )

        # ---- input DMAs: x1 on all 3 queues (scalar's triggers drain
        # before ACT's first exp), everything else on sync+gpsimd ----
        eng3 = [nc.sync, nc.scalar, nc.gpsimd]
        k = 0
        for q in range(4):
            cols = slice(q * 1024, (q + 1) * 1024)
            for c in range(4):
                eng3[k % 3].dma_start(out=x1c[c][:, cols],
                                      in_=x1_8[c * 128:(c + 1) * 128, cols])
                k += 1
            if q == 0:
                # tpw2 needed from stage1 block 4 on
                for c in range(4):
                    eng3[(k + c) % 3].dma_start(
                        out=tpw2_sb[:, c, :], in_=tpw2[c * 128:(c + 1) * 128, :])
        eng2 = [nc.sync, nc.gpsimd]
        for c in range(4):
            eng2[c % 2].dma_start(out=gw_sb[:, c, :],
                                  in_=gw8[c * 128:(c + 1) * 128, :])
        k = 0
        for q in range(4):
            cols = slice(q * 1024, (q + 1) * 1024)
            for c in range(4):
                eng2[k % 2].dma_start(out=x0c[c][:, cols],
                                      in_=x0_8[c * 128:(c + 1) * 128, cols])
                k += 1
        for oc in range(2):
            eng2[oc % 2].dma_start(out=x0h_sb[oc][:],
                                   in_=x0h[oc * 128:(oc + 1) * 128, :])
        # late-needed small tensors at the queue tails
        nc.sync.dma_start(out=id65_sb[:], in_=id65[:])
        nc.gpsimd.dma_start(out=id128_sb[:], in_=id128[:])
        nc.gpsimd.dma_start(out=W_sb[:], in_=Wext[:])
        nc.gpsimd.dma_start(out=ms_sb[:], in_=msel[:])
        nc.sync.dma_start(out=Wb_sb[:], in_=Wb2[:])
        nc.sync.dma_start(out=gb_sb[:], in_=gbc[:])

        # ---- stage 1: x1 -> theta/phi (fp8 weights, x16 prescale) ----
        with tc.tile_pool(name="ps_tp", bufs=2, space="PSUM") as ps_tp:
            for b in range(8):
                cols = slice(b * 512, (b + 1) * 512)
                tpw_sb = tpw1_sb if b < 4 else tpw2_sb
                ptp = ps_tp.tile([128, 512], FP32, name="ptp")
                for c in range(4):
                    nc.tensor.matmul(ptp[:], tpw_sb[:, c, :], x1c[c][:, cols],
                                     start=(c == 0), stop=(c == 3))
                trows = slice(0, 64) if b < 4 else slice(64, 128)
                prows = slice(64, 128) if b < 4 else slice(0, 64)
                tpb_sb = tpb1_sb if b < 4 else tpb2_sb
                ch = (b % 4) * 4
                # theta (full block) on DVE: (psum * 1/16) + bias
                nc.vector.tensor_scalar(
                    thAB[trows, ch:ch + 4, :].rearrange("p a b -> p (a b)"),
                    ptp[trows, :], IWS, tpb_sb[trows, :],
                    ALU.mult, ALU.add)
                # phi (own strided queries, compacted) on DVE
                nc.vector.tensor_scalar(
                    ph2[prows, b * 256:(b + 1) * 256].rearrange(
                        "p (g k) -> p g k", k=32),
                    ptp[prows, :].rearrange("p (g k) -> p g k", k=64)[
                        :, :, 32 * h:32 * h + 32],
                    IWS, tpb_sb[prows, :], ALU.mult, ALU.add)

        # phi lives at rows 64-127 for blocks 0-3, rows 0-63 for blocks 4-7;
        # duplicate each half to the other partition range (SBUF-SBUF DMA)
        nc.scalar.dma_start(out=ph2[0:64, 0:1024], in_=ph2[64:128, 0:1024])
        nc.scalar.dma_start(out=ph2[64:128, 1024:2048], in_=ph2[0:64, 1024:2048])

        # ---- gT: x0 chunks stationary (fp8, FWL), gw moving ----
        with tc.tile_pool(name="ps_g", bufs=3, space="PSUM") as ps_g:
            for mj in range(16):
                pg = ps_g.tile([128, 2, C8], FP32, name="pg")
                for half in range(2):
                    mi = mj * 2 + half
                    for c in range(4):
                        nc.tensor.matmul(pg[:, half, :],
                                         x0c[c][:, mi * 128:(mi + 1) * 128],
                                         gw_sb[:, c, :],
                                         start=(c == 0), stop=(c == 3))
                nc.vector.tensor_scalar(
                    g_ext[:, mj * 2:mj * 2 + 2, 0:C8], pg[:], IWS, None,
                    ALU.mult)

        # ---- main attention loop ----
        stat = ctx.enter_context(tc.tile_pool(name="stats", bufs=1))
        xst = stat.tile([128, 2, 8, 6], FP32)
        xagg = stat.tile([128, 2, 2], FP32)

        def emit_fwd_T(j, ps_t, ystage):
            ptile = ps_t.tile([128, C8 + 1], FP32, name="ptile", tag="pt")
            nc.tensor.transpose(ptile[:], yraw[:, j * 128:(j + 1) * 128],
                                id65_sb[:])
            rec = ystage.tile([128, 1], FP32, name="rec", tag="rec")
            nc.vector.reciprocal(rec[:], ptile[:, C8:C8 + 1])
            nc.vector.tensor_scalar(ynx[:, j, :], ptile[:, 0:C8], rec[:],
                                    None, ALU.mult)

        def emit_back_T(j, ps_t2):
            pt2 = ps_t2.tile([C8, 128], FP32, name="pt2", tag="pt2")
            nc.tensor.transpose(pt2[:], ynx[:, j, :], id128_sb[:])
            nc.vector.tensor_scalar(
                yn[:, j * 128:(j + 1) * 128], pt2[:], gb_sb[:], None,
                ALU.add)

        with tc.tile_pool(name="ps_fA", bufs=2, space="PSUM") as ps_fA, \
             tc.tile_pool(name="ps_fB", bufs=2, space="PSUM") as ps_fB, \
             tc.tile_pool(name="ps_y", bufs=1, space="PSUM") as ps_y, \
             tc.tile_pool(name="ps_tL", bufs=1, space="PSUM") as ps_tL, \
             tc.tile_pool(name="ps_t2L", bufs=1, space="PSUM") as ps_t2L, \
             tc.tile_pool(name="ystageL", bufs=4) as ystageL, \
             tc.tile_pool(name="ptA", bufs=4) as ptA_pool, \
             tc.tile_pool(name="ptB", bufs=4) as ptB_pool:
            for pss in range(2):
                py = ps_y.tile([C8 + 1, 2, 512], FP32, name="py")

                def emit_y(args):
                    p, pas, pbs = args
                    for s in range(2):
                        nc.tensor.matmul(py[:, s, :], g_ext[:, p, 0:C8 + 1],
                                         pas[s][:], start=(p == 0), stop=False)
                    for s in range(2):
                        nc.tensor.matmul(py[:, s, :], g_ext[:, p + 16, 0:C8 + 1],
                                         pbs[s][:],
                                         start=False, stop=(p == 15))

                prev = None
                for p in range(16):
                    fas, fbs, pas, pbs = [], [], [], []
                    for s in range(2):
                        qs = slice(pss * 1024 + s * 512,
                                   pss * 1024 + (s + 1) * 512)
                        fa = ps_fA.tile([128, 512], FP32, name="fa", tag="fa")
                        nc.tensor.matmul(fa[:], thAB[0:64, p, :], ph2[0:64, qs],
                                         start=True, stop=True)
                        fas.append(fa)
                    for s in range(2):
                        qs = slice(pss * 1024 + s * 512,
                                   pss * 1024 + (s + 1) * 512)
                        fb = ps_fB.tile([128, 512], FP32, name="fb", tag="fb")
                        nc.tensor.matmul(fb[:], thAB[64:128, p, :],
                                         ph2[64:128, qs],
                                         start=True, stop=True)
                        fbs.append(fb)
                    for s in range(2):
                        pa = ptA_pool.tile([128, 512], BF16, name="pa", tag="pa")
                        nc.scalar.activation(pa[:], fas[s][:], ACTF.Exp)
                        pas.append(pa)
                        pb = ptB_pool.tile([128, 512], BF16, name="pb", tag="pb")
                        if s == 0:
                            nc.vector.tensor_scalar(pb[:].bitcast(I16),
                                                    fbs[s][:], EXPA, EXPB,
                                                    ALU.mult, ALU.add)
                        else:
                            # rebalance: DVE half, ACT half
                            nc.vector.tensor_scalar(
                                pb[:, 0:256].bitcast(I16), fbs[s][:, 0:256],
                                EXPA, EXPB, ALU.mult, ALU.add)
                            nc.scalar.activation(pb[:, 256:512],
                                                 fbs[s][:, 256:512], ACTF.Exp)
                        pbs.append(pb)
                    # overlap pss0-chunk transposes into pss1
                    if pss == 1 and p % 2 == 1:
                        emit_fwd_T(p // 2, ps_tL, ystageL)
                    if pss == 1 and p % 2 == 0 and p >= 2:
                        emit_back_T(p // 2 - 1, ps_t2L)
                    # keep the CC stack hot: data-tied dummy mid-pss1
                    if pss == 1 and p == 8:
                        nc.scalar.dma_start(out=ccw3_in[:],
                                            in_=pas[0][0:1, 0:16])
                        nc.gpsimd.collective_compute(
                            "AllReduce", ALU.add,
                            replica_groups=REPLICA_PAIRS,
                            ins=[ccw3_in[:]], outs=[ccw3_out[:]])
                    if prev is not None:
                        emit_y(prev)
                    prev = (p, pas, pbs)
                emit_y(prev)
                if pss == 1:
                    emit_back_T(7, ps_t2L)

                nc.vector.tensor_copy(
                    yraw[:, pss * 1024:(pss + 1) * 1024],
                    py[:].rearrange("p a b -> p (a b)"))
                nc.scalar.dma_start(
                    out=ccw2_in[:] if pss == 0 else ccw4_in[:],
                    in_=yraw[C8:C8 + 1, pss * 1024:pss * 1024 + 16])
                nc.gpsimd.collective_compute(
                    "AllReduce", ALU.add,
                    replica_groups=REPLICA_PAIRS,
                    ins=[ccw2_in[:] if pss == 0 else ccw4_in[:]],
                    outs=[ccw2_out[:] if pss == 0 else ccw4_out[:]])

        # ---- transpose remaining yT chunks (8-15) ----
        with tc.tile_pool(name="ps_t", bufs=2, space="PSUM") as ps_t, \
             tc.tile_pool(name="ps_t2", bufs=2, space="PSUM") as ps_t2, \
             tc.tile_pool(name="ystage", bufs=4) as ystage:
            for j in range(8, 16):
                emit_fwd_T(j, ps_t, ystage)
                emit_back_T(j, ps_t2)

        # ---- K = sum_a yn_a yn_a^T (+ mean col via ones), AllReduce ----
        yn_r = yn[:].rearrange("p (c a) -> p c a", a=32)
        with tc.tile_pool(name="ps_k", bufs=1, space="PSUM") as ps_k, \
             tc.tile_pool(name="kst", bufs=1) as kst:
            kps = ps_k.tile([C8, C8 + 1], FP32, name="kps")
            for a in range(32):
                nc.tensor.matmul(
                    kps[:],
                    yn_r[:, 0:C8, a:a + 1].rearrange("p c o -> p (c o)"),
                    yn_r[:, 0:C8 + 1, a:a + 1].rearrange("p c o -> p (c o)"),
                    start=(a == 0), stop=(a == 31))
            ksb = kst.tile([C8, C8 + 1], FP32, name="ksb")
            nc.vector.tensor_copy(ksb[:], kps[:])

            # ---- per-channel S1 = w^T K_own w, S2 = m_own^T w (linear in
            # K => AllReduce the [128,4] scalars instead of K itself) ----
            with tc.tile_pool(name="ps_kw", bufs=1, space="PSUM") as ps_kw, \
                 tc.tile_pool(name="sc", bufs=1) as sc:
                # partial S for ALL 512 out channels in GLOBAL order so
                # the pairwise AllReduce adds matching quadratic forms;
                # S1 = column sums of P via ones^T @ P, S2 = P row 64
                kw = ps_kw.tile([C8 + 1, C], FP32, name="kw")
                nc.tensor.matmul(kw[:], ksb[:], W_sb[0:C8, :],
                                 start=True, stop=True)
                P_sb = sc.tile([C8 + 1, C], BF16)
                nc.vector.tensor_mul(P_sb[:], kw[:], W_sb[:])
                s1ps = ps_kw.tile([1, C], FP32, name="s1ps")
                nc.tensor.matmul(s1ps[:], ones64[:], P_sb[0:C8, :],
                                 start=True, stop=True)
                s1sb = sc.tile([1, C], FP32, name="s1sb")
                nc.vector.tensor_copy(s1sb[:], s1ps[:])
                nc.sync.dma_start(out=sin[0:1, :], in_=s1sb[:])
                nc.gpsimd.dma_start(out=sin[1:2, :], in_=P_sb[C8:C8 + 1, :])
                nc.gpsimd.collective_compute(
                    "AllReduce", ALU.add,
                    replica_groups=REPLICA_PAIRS,
                    ins=[sin[:]],
                    outs=[sout[:]],
                )
                # x0 instance stats on DVE during the collective wait;
                # tile_wait_until keeps the scheduler from hoisting these
                # ahead of the stage1/loop DVE work (their only data dep is
                # the x0h input DMA, which lands early)
                vc = sc.tile([128, 2], FP32)
                rc = sc.tile([128, 2], FP32)
                with tc.tile_wait_until(0.125):
                    for oc in range(2):
                        for mb in range(8):
                            nc.vector.bn_stats(
                                xst[:, oc, mb, :],
                                x0h_sb[oc][:, mb * 512:(mb + 1) * 512])
                    for oc in range(2):
                        nc.vector.bn_aggr(xagg[:, oc, :], xst[:, oc, :, :])
                    nc.vector.tensor_scalar_add(vc[:], xagg[:, :, 1], EPS)
                    nc.vector.reciprocal(rc[:], vc[:])
                    # pre-switch the ACT table Exp->Sqrt while CC runs
                    warm2 = sc.tile([128, 1], FP32, name="warm2")
                    nc.scalar.activation(warm2[:], tpb1_sb[:], ACTF.Sqrt)
                # readback reshaped: Sred[p, r, g] = sout[r, g*128+p]
                Sred = sc.tile([128, 2, 4], FP32)
                nc.sync.dma_start(
                    out=Sred[:],
                    in_=sout[:].rearrange("r (g p) -> p r g", p=128))

                # select own half's channels: msel holds invN (own) / 0,
                # so E2 = S1_own/N and mu0 = S2_own/N after mask-add
                e1 = sc.tile([128, 2, 2], FP32)
                nc.vector.tensor_scalar(e1[:], Sred[:, :, 0:2],
                                        ms_sb[:, 0:1], None, ALU.mult)
                e2b = sc.tile([128, 2, 2], FP32)
                nc.vector.tensor_scalar(e2b[:], Sred[:, :, 2:4],
                                        ms_sb[:, 1:2], None, ALU.mult)
                Eall = sc.tile([128, 2, 2], FP32)
                nc.vector.tensor_add(Eall[:], e1[:], e2b[:])
                E2 = Eall[:, 0, :]
                mu0 = Eall[:, 1, :]
                mus = sc.tile([128, 2], FP32)
                nc.vector.tensor_add(mus[:], mu0, Wb_sb[:])
                m2 = sc.tile([128, 2], FP32)
                nc.vector.tensor_mul(m2[:], mu0, mu0)
                vs = sc.tile([128, 2], FP32)
                nc.vector.tensor_sub(vs[:], E2, m2[:])
                nc.vector.tensor_scalar_add(vs[:], vs[:], EPS)
                ratio = sc.tile([128, 2], FP32)
                nc.vector.tensor_mul(ratio[:], vs[:], rc[:])
                # r = sqrt(ratio) on ACT (table pre-switched)
                rr = sc.tile([128, 2], FP32)
                nc.scalar.activation(rr[:], ratio[:], ACTF.Sqrt)
                rmc = sc.tile([128, 2], FP32)
                nc.vector.tensor_mul(rmc[:], rr[:], xagg[:, :, 0])
                tt = sc.tile([128, 2], FP32)
                nc.vector.tensor_sub(tt[:], mus[:], rmc[:])

                # ---- out = r * x0 + t, split across engines/queues ----
                with tc.tile_pool(name="outp", bufs=6) as outp:
                    deng = [nc.sync, nc.gpsimd, nc.scalar]
                    kinds = [1, 0, 1, 2, 1, 0, 1, 2]
                    for ocn in range(2):
                        for mb in range(4):
                            cols = slice(mb * 1024, (mb + 1) * 1024)
                            ot = outp.tile([128, 1024], FP16, name="ot",
                                           tag="ot")
                            kind = kinds[ocn * 4 + mb]
                            if kind == 0:
                                nc.gpsimd.tensor_scalar(
                                    ot[:], x0h_sb[ocn][:, cols],
                                    rr[:, ocn:ocn + 1], tt[:, ocn:ocn + 1],
                                    ALU.mult, ALU.add)
                            elif kind == 1:
                                nc.vector.tensor_scalar(
                                    ot[:], x0h_sb[ocn][:, cols],
                                    rr[:, ocn:ocn + 1], tt[:, ocn:ocn + 1],
                                    ALU.mult, ALU.add)
                            else:
                                nc.scalar.activation(
                                    ot[:], x0h_sb[ocn][:, cols],
                                    ACTF.Identity,
                                    bias=tt[:, ocn:ocn + 1],
                                    scale=rr[:, ocn:ocn + 1])
                            deng[(ocn * 4 + mb) % 3].dma_start(
                                out=out[ocn * 128:(ocn + 1) * 128, cols],
                                in_=ot[:])

    _split_excess_waits(nc)
    return nc


_NC_CACHE = None


def _get_nc():
    global _NC_CACHE
    if _NC_CACHE is None:
        _NC_CACHE = build_nc()
    return _NC_CACHE


def _roll32(xf):
    """Roll each 64-token group by 32: token 64c+a -> 64c+((a+32)%64).
    Output position p holds input token 64*(p//64) + (p%64+32)%64."""
    v = xf.reshape(xf.shape[0], N // 64, 2, 32)
    return np.ascontiguousarray(
        np.concatenate([v[:, :, 1, :], v[:, :, 0, :]], axis=2).reshape(
            xf.shape[0], N))


def make_in_maps(x0, x1, g_w, g_b, theta_w, theta_b, phi_w, phi_b, W_w, W_b):
    f8 = mybir.dt.np(FP8)
    x0f = np.asarray(x0, np.float32).reshape(B, C, N)
    x1f = np.asarray(x1, np.float32).reshape(B, C, N)
    tw = np.asarray(theta_w, np.float32).T * WS       # [C, C8]
    pw = np.asarray(phi_w, np.float32).T * WS
    tpw1 = np.ascontiguousarray(np.concatenate([tw, pw], axis=1)).astype(f8)
    tpw2 = np.ascontiguousarray(np.concatenate([pw, tw], axis=1)).astype(f8)
    tb = np.asarray(theta_b, np.float32)
    pb = np.asarray(phi_b, np.float32)
    tpb1 = np.ascontiguousarray(np.concatenate([tb, pb])[:, None])
    tpb2 = np.ascontiguousarray(np.concatenate([pb, tb])[:, None])
    gw8 = np.ascontiguousarray(np.asarray(g_w, np.float32).T * WS).astype(f8)
    W_wf = np.asarray(W_w, np.float32)                # [C, C8]
    W_bf = np.asarray(W_b, np.float32)
    id65 = np.eye(C8 + 1, dtype=np.float32)
    id128 = np.eye(128, dtype=np.float32)
    gbc = np.ascontiguousarray(np.asarray(g_b, np.float32)[:, None])

    in_maps = []
    for core in range(8):
        b, half = core // 2, core % 2
        Wext = np.concatenate(
            [W_wf.T, np.ones((1, C), np.float32)], axis=0)  # [65, 512] global
        msel = np.zeros((128, 2), np.float32)
        msel[:, half] = 1.0 / N
        x1b = x1f[b] if half == 0 else _roll32(x1f[b])
        x0b = x0f[b] if half == 0 else _roll32(x0f[b])
        in_maps.append({
            "x1_8": x1b.astype(f8),
            "x0_8": x0b.astype(f8),
            "x0h": np.ascontiguousarray(
                x0f[b][half * OC:(half + 1) * OC]).astype(np.float16),
            "tpw1": tpw1,
            "tpw2": tpw2,
            "tpb1": tpb1,
            "tpb2": tpb2,
            "gw8": gw8,
            "Wext": np.ascontiguousarray(Wext),
            "msel": msel,
            "Wb2": np.ascontiguousarray(
                W_bf[half * OC:(half + 1) * OC].reshape(2, 128).T),
            "gbc": gbc,
            "id65": id65,
            "id128": id128,
        })
    return in_maps


def kernel(x0, x1, g_w, g_b, theta_w, theta_b, phi_w, phi_b, W_w, W_b):
    in_maps = make_in_maps(x0, x1, g_w, g_b, theta_w, theta_b, phi_w, phi_b,
                           W_w, W_b)
    nc = _get_nc()
    res = run_bass_kernel_spmd(nc, in_maps, core_ids=list(range(8)))

    outf = np.empty((B, C, N), dtype=np.float32)
    for core in range(8):
        b, half = core // 2, core % 2
        o = np.asarray(res.results[core]["out"], dtype=np.float32)
        outf[b, half * OC:(half + 1) * OC] = o
    return outf.reshape(B, C, H, W)
